# revision 1
# baseline (speedup 1.0000x reference)
"""Trainium2 Bass kernel for CRFHead (dense-Gaussian mean-field CRF).

Problem (hardcoded): B=2 images, 3x96x96, masks 96x96. Per image:
  - quantize image/mask to uint8-valued floats
  - unary from normalized mask; bilateral features (x/60, y/60, rgb/5)
  - exact dense Gaussian kernel K = exp(-0.5 |f_i - f_j|^2), N = 9216
  - symmetric normalization Kn = D^-1/2 K D^-1/2
  - 10 mean-field iterations Q <- softmax(-U + 5 * Kn Q); output Q[:,1]

Device algorithm (per core; 8 cores = 2 images x 4-way row sharding):
  Binary-class softmax reduces to q' = sigmoid(logit(U) + 5*(2*Kn q - Kn 1)).
  With p = q/sqrt(deg):  Kn q|_i = (1/sqrt(deg_i)) * sum_j K_ij p_j, so each
  pass computes R_i = sum_j exp(-0.5 d2_ij + ln p_j).

  The exponent is built by ONE K=12 bf16 matmul per tile:
    - rgb dims (3): raw integer pixel values (exact in bf16)
    - xy dims (6): hi/lo split of x/12, y/12 (cross terms to ~1e-5)
    - w dims (3): hi/mid/lo bf16 split of the per-j additive term
        w_j = 25*(ln p_j - sqrgb_j/50 - sqxy_j/7200)
  then ACT: exp(PSUM/25 + bias_i) with per-partition bias
    bias_i = -sqrgb_i/50 - sqxy_i/7200,
  accumulated along the free dim by the ACT accumulator.

  12 passes total: deg, t = K @ (1/sqrt(deg)), and 10 iterations. Between
  iterations the 3 bf16 w-rows are AllGathered (4-core groups) via DRAM.
"""

import numpy as np
import ml_dtypes

B, C, H, W = 2, 3, 96, 96
N = H * W            # 9216 pixels per image
N_CORES = 8
SHARDS = 4           # cores per image
ROWS = N // SHARDS   # 2304 local rows per core
NT = ROWS // 128     # 18 i-tiles per core
KDIM = 12
CHUNKS = [(0, 2048), (2048, 2048), (4096, 2048), (6144, 2048), (8192, 1024)]
NCH = len(CHUNKS)
REFINE_ITERS = 10
RG = [[0, 1, 2, 3], [4, 5, 6, 7]]

BF = ml_dtypes.bfloat16


def _bf(x):
    return np.asarray(x, dtype=BF).astype(np.float32)


def _split3(w):
    """3-way bf16 split of fp32 vector w (sum of parts ~= w)."""
    w = np.asarray(w, np.float32)
    w1 = np.asarray(w, BF)
    d1 = w - w1.astype(np.float32)
    w2 = np.asarray(d1, BF)
    w3 = np.asarray(d1 - w2.astype(np.float32), BF)
    return w1, w2, w3


def _host_prep(imgs, masks):
    """Mirror the reference's quantization exactly in numpy fp32."""
    imgs = np.asarray(imgs, np.float32)
    masks = np.asarray(masks, np.float32)
    MEAN = np.array([0.485, 0.456, 0.406], np.float32)[None, :, None, None]
    STD = np.array([0.229, 0.224, 0.225], np.float32)[None, :, None, None]
    x = (imgs * STD + MEAN).transpose(0, 2, 3, 1) * np.float32(255.0)
    x = np.floor(np.clip(x, 0.0, 255.0))
    m = np.floor(np.clip(masks * np.float32(255.0) / np.float32(0.7), 0.0, 255.0))
    return x, m


def _image_data(img_q, mask_q):
    """Per-image full-N host arrays for the device program."""
    U = mask_q / (mask_q.max() + 1e-8)
    U = np.clip(U, 1e-6, 1.0 - 1e-6).reshape(N).astype(np.float32)
    logitU = np.log(U / (np.float32(1.0) - U)).astype(np.float32)

    ys, xs = np.meshgrid(np.arange(H, dtype=np.float32),
                         np.arange(W, dtype=np.float32), indexing="ij")
    xv = xs.reshape(N)
    yv = ys.reshape(N)
    c = img_q.reshape(N, 3).astype(np.float32)

    ax = (xv / np.float32(12.0)).astype(np.float32)
    ay = (yv / np.float32(12.0)).astype(np.float32)
    axh = _bf(ax); axl = _bf(ax - axh)
    ayh = _bf(ay); ayl = _bf(ay - ayh)
    r, g, b = _bf(c[:, 0]), _bf(c[:, 1]), _bf(c[:, 2])
    ones = np.ones(N, np.float32)

    sqxy = xv * xv + yv * yv
    sqrgb = (c * c).sum(axis=1)
    bias = (-sqrgb / np.float32(50.0) - sqxy / np.float32(7200.0)).astype(np.float32)
    wstat = (np.float32(25.0) * bias).astype(np.float32)

    i_rows = np.stack([r, g, b, axh, axh, axl, ayh, ayh, ayl, ones, ones, ones])
    j_static = np.stack([r, g, b, axh, axl, axh, ayh, ayl, ayh]).astype(BF)
    wsplits = np.stack(_split3(wstat))  # (3, N) bf16

    return dict(U=U, logitU=logitU, bias=bias, wstat=wstat,
                i_rows=i_rows, j_static=j_static, wsplits=wsplits)


def _core_inputs(data, g):
    """Slice per-image data for shard g into device layout.

    Local row (p, t) <-> global row g*ROWS + p*NT + t.
    lhsT column t*128 + p <-> same global row (i-tile t puts row at psum
    partition p).
    """
    g_idx = g * ROWS + np.arange(128)[:, None] * NT + np.arange(NT)[None, :]
    tt, pp = np.meshgrid(np.arange(NT), np.arange(128), indexing="ij")
    col_rows = (g * ROWS + pp * NT + tt).reshape(-1)

    vecs = np.stack([data["bias"][g_idx], data["logitU"][g_idx],
                     data["U"][g_idx], data["wstat"][g_idx]],
                    axis=1).reshape(128, 4 * NT)  # [128, 4*NT], blocks of NT
    # stack axis=1 gives [128, 4, NT] -> reshape packs (kind, t) contiguously
    return {
        "lhsT": data["i_rows"][:, col_rows].astype(BF),
        "rhs_static": np.ascontiguousarray(np.asarray(data["j_static"], BF)),
        "wstat_splits": np.ascontiguousarray(np.asarray(data["wsplits"], BF)),
        "vecs": np.ascontiguousarray(vecs, np.float32),
    }


def build_program():
    import concourse.bacc as bacc
    import concourse.mybir as mybir
    from concourse.tile import TileContext

    f32 = mybir.dt.float32
    bf16 = mybir.dt.bfloat16
    AF = mybir.ActivationFunctionType

    nc = bacc.Bacc(num_devices=N_CORES)

    lhsT_in = nc.dram_tensor("lhsT", [KDIM, ROWS], bf16, kind="ExternalInput")
    rhs_in = nc.dram_tensor("rhs_static", [9, N], bf16, kind="ExternalInput")
    wsplits_in = nc.dram_tensor("wstat_splits", [3, N], bf16,
                                kind="ExternalInput")
    vecs_in = nc.dram_tensor("vecs", [128, 4 * NT], f32, kind="ExternalInput")
    q_out_d = nc.dram_tensor("q_out", [128, NT], f32, kind="ExternalOutput")

    with TileContext(nc) as tc:
        with (
            tc.tile_pool(name="const", bufs=1) as cpool,
            tc.tile_pool(name="vec", bufs=2) as vp,
            tc.tile_pool(name="esc", bufs=3) as ep,
            tc.tile_pool(name="racc", bufs=2) as rp,
            tc.tile_pool(name="psum", bufs=2, space="PSUM") as pp,
            tc.tile_pool(name="dram", bufs=2, space="DRAM") as dp,
        ):
            # --- persistent SBUF state ---
            rhs_sb = cpool.tile([KDIM, N], bf16, tag="rhs")
            lhsT_sb = cpool.tile([KDIM, ROWS], bf16, tag="lhsT")
            vecs_sb = cpool.tile([128, 4 * NT], f32, tag="vecs")
            bias_sb = vecs_sb[:, 0:NT]
            logitU_sb = vecs_sb[:, NT:2 * NT]
            U_sb = vecs_sb[:, 2 * NT:3 * NT]
            wstat_sb = vecs_sb[:, 3 * NT:4 * NT]
            deg_sb = cpool.tile([128, NT], f32, tag="deg")
            ln_deg = cpool.tile([128, NT], f32, tag="ln_deg")
            rsqd = cpool.tile([128, NT], f32, tag="rsqd")
            wstat2 = cpool.tile([128, NT], f32, tag="wstat2")
            Acoef = cpool.tile([128, NT], f32, tag="Acoef")
            Ccoef = cpool.tile([128, NT], f32, tag="Ccoef")
            tvec = cpool.tile([128, NT], f32, tag="tvec")

            nc.sync.dma_start(out=rhs_sb[0:9, :], in_=rhs_in[:, :])
            nc.sync.dma_start(out=rhs_sb[9:12, :], in_=wsplits_in[:, :])
            nc.sync.dma_start(out=lhsT_sb[:, :], in_=lhsT_in[:, :])
            nc.sync.dma_start(out=vecs_sb[:, :], in_=vecs_in[:, :])

            SCALE = float(np.float32(1.0) / np.float32(25.0))

            def kernel_pass(out_R):
                """R_i = sum_j exp(PSUM/25 + bias_i); returns [128, NT] f32.

                One K=12 bf16 matmul per 512-slice (weights stay loaded
                across each i-tile); exp on ACT; free-dim reduction on the
                otherwise-idle Vector engine."""
                racc = rp.tile([128, NT, NCH], f32, tag="racc")
                for t in range(NT):
                    lhs = lhsT_sb[:, t * 128:(t + 1) * 128]
                    for ci, (j0, jw) in enumerate(CHUNKS):
                        ps = pp.tile([128, 2048], f32, tag="ps")
                        for s0 in range(0, jw, 512):
                            nc.tensor.matmul(
                                ps[:, s0:s0 + 512],
                                lhs,
                                rhs_sb[:, j0 + s0:j0 + s0 + 512],
                                start=True, stop=True,
                            )
                        esc = ep.tile([128, 2048], f32, tag="esc")
                        if ci == NCH - 1:
                            # rebalance: short chunk's reduction rides the
                            # ACT accumulator; DVE keeps the wide chunks
                            nc.scalar.activation(
                                esc[:, :jw], ps[:, :jw], AF.Exp,
                                bias=bias_sb[:, t:t + 1], scale=SCALE,
                                accum_out=racc[:, t, ci:ci + 1],
                            )
                        else:
                            nc.scalar.activation(
                                esc[:, :jw], ps[:, :jw], AF.Exp,
                                bias=bias_sb[:, t:t + 1], scale=SCALE,
                            )
                            nc.vector.tensor_reduce(
                                racc[:, t, ci:ci + 1], esc[:, :jw],
                                mybir.AxisListType.X, mybir.AluOpType.add,
                            )
                nc.vector.tensor_reduce(
                    out_R[:, :], racc[:, :, :], mybir.AxisListType.X,
                    mybir.AluOpType.add,
                )

            def load_w_rows(wsrc):
                """Split wsrc [128,NT] f32 into 3 bf16 components, allgather
                over the 4-core group, and rewrite rhs_sb rows 9..11."""
                wbs = []
                cur = wsrc
                for sp in range(3):
                    wb = vp.tile([128, NT], bf16, tag=f"wb{sp}")
                    nc.vector.tensor_copy(wb[:, :], cur[:, :])
                    wbs.append(wb)
                    if sp < 2:
                        wf = vp.tile([128, NT], f32, tag=f"wf{sp}")
                        nc.vector.tensor_copy(wf[:, :], wb[:, :])
                        nxt = vp.tile([128, NT], f32, tag=f"wd{sp}")
                        nc.vector.tensor_sub(nxt[:, :], cur[:, :], wf[:, :])
                        cur = nxt
                wsp = dp.tile([3, ROWS], bf16, tag="wsplit")
                for sp in range(3):
                    nc.sync.dma_start(out=wsp[sp, :], in_=wbs[sp][:, :])
                wg = dp.tile([SHARDS, 3, ROWS], bf16, tag="wgath")
                nc.gpsimd.collective_compute(
                    "AllGather",
                    mybir.AluOpType.bypass,
                    replica_groups=RG,
                    ins=[wsp[:].opt()],
                    outs=[wg[:].opt()],
                )
                for sp in range(3):
                    nc.sync.dma_start(out=rhs_sb[9 + sp:10 + sp, :],
                                      in_=wg[:, sp, :])

            # --- pass 1: deg (w rows preloaded with wstat splits) ---
            kernel_pass(deg_sb)
            nc.scalar.activation(ln_deg[:, :], deg_sb[:, :], AF.Ln)
            nc.scalar.activation(rsqd[:, :], ln_deg[:, :], AF.Exp, scale=-0.5)
            tmp = vp.tile([128, NT], f32, tag="tmp")
            nc.vector.tensor_scalar_mul(tmp[:, :], ln_deg[:, :], -12.5)
            nc.vector.tensor_add(wstat2[:, :], wstat_sb[:, :], tmp[:, :])

            # --- pass 2: tvec = sum_j K_ij / sqrt(deg_j) ---
            load_w_rows(wstat2)
            kernel_pass(tvec)

            # update coefficients: z = C + A*R with
            #   A = 10 / sqrt(deg_i),  C = logitU - 5 * tvec / sqrt(deg_i)
            nc.vector.tensor_scalar_mul(Acoef[:, :], rsqd[:, :], 10.0)
            tmp2 = vp.tile([128, NT], f32, tag="tmp2")
            nc.vector.tensor_mul(tmp2[:, :], rsqd[:, :], tvec[:, :])
            tmp3 = vp.tile([128, NT], f32, tag="tmp3")
            nc.vector.tensor_scalar_mul(tmp3[:, :], tmp2[:, :], -5.0)
            nc.vector.tensor_add(Ccoef[:, :], logitU_sb[:, :], tmp3[:, :])

            # --- 10 mean-field iterations ---
            # carry the iteration through e = exp(-z):
            #   q = 1/(1+e)  and  ln q = -ln(1+e), so the next w is
            #   wv = -25*Ln(e + 1) + wstat2 (bias folds the +1 into ACT);
            #   q itself is only materialized after the last pass.
            lq = vp.tile([128, NT], f32, tag="lq")
            nc.scalar.activation(lq[:, :], U_sb[:, :], AF.Ln)
            t1 = vp.tile([128, NT], f32, tag="t1")
            nc.vector.tensor_scalar_mul(t1[:, :], lq[:, :], 25.0)
            wv = vp.tile([128, NT], f32, tag="wv")
            nc.vector.tensor_add(wv[:, :], t1[:, :], wstat2[:, :])
            e = None
            for it in range(REFINE_ITERS):
                if it > 0:
                    lqn = vp.tile([128, NT], f32, tag="lqn")
                    nc.scalar.activation(lqn[:, :], e[:, :], AF.Ln, bias=1.0)
                    t1 = vp.tile([128, NT], f32, tag="t1")
                    nc.vector.tensor_scalar_mul(t1[:, :], lqn[:, :], -25.0)
                    wv = vp.tile([128, NT], f32, tag="wv")
                    nc.vector.tensor_add(wv[:, :], t1[:, :], wstat2[:, :])
                load_w_rows(wv)
                R = vp.tile([128, NT], f32, tag="R")
                kernel_pass(R)
                t2 = vp.tile([128, NT], f32, tag="t2")
                nc.vector.tensor_mul(t2[:, :], Acoef[:, :], R[:, :])
                z = vp.tile([128, NT], f32, tag="z")
                nc.vector.tensor_add(z[:, :], Ccoef[:, :], t2[:, :])
                e = vp.tile([128, NT], f32, tag="e")
                nc.scalar.activation(e[:, :], z[:, :], AF.Exp, scale=-1.0)
            e1 = vp.tile([128, NT], f32, tag="e1")
            nc.vector.tensor_scalar_add(e1[:, :], e[:, :], 1.0)
            q = vp.tile([128, NT], f32, tag="qn")
            nc.vector.reciprocal(q[:, :], e1[:, :])

            nc.sync.dma_start(out=q_out_d[:, :], in_=q[:, :])

    nc.compile()
    _dedup_act_table_loads(nc, mybir)
    return nc


def _dedup_act_table_loads(nc, mybir):
    """All our ACT funcs (Exp, Ln, Copy, Identity) live in one table set;
    keep a single load of that set instead of per-switch reloads."""
    from concourse.hw_specs import get_activation_tables
    tables = list(get_activation_tables(nc.m.arch).items())
    want = {mybir.ActivationFunctionType.Exp, mybir.ActivationFunctionType.Ln}
    target = None
    for idx, (_, funcs) in enumerate(tables):
        if want <= funcs:
            target = idx
            break
    if target is None:
        return
    first = True
    for blk in nc.m.functions[0].blocks:
        il = blk.instructions
        drop = []
        for ins in il:
            if type(ins).__name__ == "InstLoadActFuncSet":
                sync = getattr(ins, "sync_info", None)
                if sync is not None and (sync.on_wait or sync.on_update):
                    continue  # carries sems; leave untouched
                if first:
                    ins.act_func_set_id = target
                    first = False
                else:
                    drop.append(ins)
        for ins in drop:
            il.remove(ins)


def make_in_maps(imgs, masks):
    x, m = _host_prep(imgs, masks)
    per_image = [_image_data(x[b], m[b]) for b in range(B)]
    in_maps = []
    for k in range(N_CORES):
        b, g = divmod(k, SHARDS)
        in_maps.append(_core_inputs(per_image[b], g))
    return in_maps


def assemble(results):
    """results: list of per-core dicts with 'q_out' [128, NT] f32."""
    out = np.empty((B, N), np.float32)
    g_idx = np.arange(128)[:, None] * NT + np.arange(NT)[None, :]
    for k in range(N_CORES):
        b, g = divmod(k, SHARDS)
        flat = np.empty(ROWS, np.float32)
        flat[g_idx.reshape(-1)] = np.asarray(results[k]["q_out"],
                                             np.float32).reshape(-1)
        out[b, g * ROWS:(g + 1) * ROWS] = flat
    return out.reshape(B, H, W)


_NC_CACHE = None


def kernel(imgs, masks):
    global _NC_CACHE
    from concourse.bass_utils import run_bass_kernel_spmd

    in_maps = make_in_maps(imgs, masks)
    if _NC_CACHE is None:
        _NC_CACHE = build_program()
    res = run_bass_kernel_spmd(_NC_CACHE, in_maps, list(range(N_CORES)))
    return assemble(res.results)



# revision 8
# speedup vs baseline: 3.8056x; 3.8056x over previous
"""Trainium2 Bass kernel for CRFHead (dense-Gaussian mean-field CRF).

Problem (hardcoded): B=2 images, 3x96x96, masks 96x96, N=9216 pixels,
10 mean-field iterations over the exact dense Gaussian kernel
K = exp(-0.5|f_i-f_j|^2), f = (x/60, y/60, rgb/5), symmetric-normalized.

Key structure exploited: with uint8 colors / sigma_rgb=5, K is ~99% tiny.
Pixels are sorted by the red channel; pairs with |dr| > T are dropped
(K <= exp(-T^2/50), negligible).  In sorted order each i-slot of 512/256
pixels only interacts with a contiguous, chunk-aligned j-window.

Per-core program (8 cores = 2 images x 4-way i-sharding, SPMD):
  - BUILD: one K=15 bf16 matmul per (slot, window-chunk) computes the
    pair exponents (features + both static |f|^2 terms ride the matmul);
    ACT exp writes the banded kernel matrix E into SBUF as fp16
    (j on partitions, i on the free dim).  ~140KB/partition, stays put.
  - deg / tvec / 10 iterations are then banded mat-vecs R = E^T w using
    M=1 matmuls (w chunk stationary, E chunks streamed), col-tiled 4x
    via tile_position.  Weights are fp16 hi+lo split pairs accumulated
    in one PSUM chain (restores f32-weight accuracy, which the
    near-chaotic mean-field dynamics require).
  - Between iterations only the tiny p = q/sqrt(deg) vector is
    exchanged (AllGather in each image's 4-core group).  Each core works
    in a local j-space = its own range +- one neighbor core; the two
    neighbor rows are fetched by an indirect DMA whose row indices are
    per-core input data, keeping the program core-uniform.

The j-window layout (chunk starts/widths per slot) is data-dependent; it
is computed on the host from the actual inputs and baked into the
compiled program (cached per window signature).
"""

import numpy as np
import ml_dtypes

B, C, H, W = 2, 3, 96, 96
N = H * W                      # 9216 pixels per image
N_CORES = 8
SHARDS = 4                     # cores per image
ROWS = N // SHARDS             # 2304 local pixels per core
TS = (512, 512, 512, 512, 256)             # i-slot sizes per core
OFF = (0, 512, 1024, 1536, 2048)           # i-slot offsets
PCOL = (0, 0, 0, 0, 512)                   # piece col block per slot
PPART = (0, 32, 64, 96, 0)                 # piece partition per slot
NSLOT = 5
LCHUNKS = 54                   # local j-space: 3 cores x 18 chunks
T_BAND = 25.0                  # red-channel band threshold
REFINE_ITERS = 10
RG = [[0, 1, 2, 3], [4, 5, 6, 7]]
KD = 15                        # matmul contraction rows

BF = ml_dtypes.bfloat16


def _bf(x):
    return np.asarray(x, dtype=BF).astype(np.float32)


def _split3(w):
    """3-way bf16 split of fp32 vector w (sum of parts ~= w)."""
    w = np.asarray(w, np.float32)
    w1 = np.asarray(w, BF)
    d1 = w - w1.astype(np.float32)
    w2 = np.asarray(d1, BF)
    w3 = np.asarray(d1 - w2.astype(np.float32), BF)
    return (w1.astype(np.float32), w2.astype(np.float32),
            w3.astype(np.float32))


def _host_prep(imgs, masks):
    """Mirror the reference's quantization exactly in numpy fp32."""
    imgs = np.asarray(imgs, np.float32)
    masks = np.asarray(masks, np.float32)
    MEAN = np.array([0.485, 0.456, 0.406], np.float32)[None, :, None, None]
    STD = np.array([0.229, 0.224, 0.225], np.float32)[None, :, None, None]
    x = (imgs * STD + MEAN).transpose(0, 2, 3, 1) * np.float32(255.0)
    x = np.floor(np.clip(x, 0.0, 255.0))
    m = np.floor(np.clip(masks * np.float32(255.0) / np.float32(0.7), 0.0, 255.0))
    return x, m


def _image_data(img_q, mask_q):
    """Sorted-order host arrays + per-(core,slot) windows for one image."""
    c = img_q.reshape(N, 3).astype(np.float32)
    perm = np.argsort(c[:, 0], kind="stable")
    rs = c[perm]

    ys, xs = np.meshgrid(np.arange(H, dtype=np.float32),
                         np.arange(W, dtype=np.float32), indexing="ij")
    xv = xs.reshape(N)[perm]
    yv = ys.reshape(N)[perm]

    U = mask_q / (mask_q.max() + np.float32(1e-8))
    U = np.clip(U, 1e-6, 1.0 - 1e-6).reshape(N).astype(np.float32)[perm]
    logitU = np.log(U / (np.float32(1.0) - U)).astype(np.float32)

    ax = (xv / np.float32(12.0)).astype(np.float32)
    ay = (yv / np.float32(12.0)).astype(np.float32)
    axh = _bf(ax); axl = _bf(ax - axh)
    ayh = _bf(ay); ayl = _bf(ay - ayh)
    r, g, b = _bf(rs[:, 0]), _bf(rs[:, 1]), _bf(rs[:, 2])
    ones = np.ones(N, np.float32)

    sqxy = xv * xv + yv * yv
    sqrgb = (rs * rs).sum(axis=1)
    wstat = (np.float32(25.0) * (-sqrgb / np.float32(50.0)
                                 - sqxy / np.float32(7200.0))).astype(np.float32)
    w1, w2, w3 = _split3(wstat)

    # j side (lhsT, partitions) and i side (rhs, free); PSUM[j,i]/25 =
    # f_j.f_i - 0.5|f_j|^2 - 0.5|f_i|^2 (modulo tiny axl*axl cross terms).
    j_rows = np.stack([r, g, b, axh, axh, axl, ayh, ayh, ayl,
                       ones, ones, ones, w1, w2, w3])
    i_rows = np.stack([r, g, b, axh, axl, axh, ayh, ayl, ayh,
                       w1, w2, w3, ones, ones, ones])

    # windows per (core, slot): chunk-aligned [rmin-T, rmax+T] in sorted r
    rsort = rs[:, 0]
    lo = np.empty((SHARDS, NSLOT), np.int64)
    hi = np.empty((SHARDS, NSLOT), np.int64)
    for g_ in range(SHARDS):
        for s in range(NSLOT):
            a = g_ * ROWS + OFF[s]
            bnd = a + TS[s]
            jlo = np.searchsorted(rsort, rsort[a] - T_BAND, side="left")
            jhi = np.searchsorted(rsort, rsort[bnd - 1] + T_BAND, side="right")
            lo[g_, s] = jlo // 128
            hi[g_, s] = -(-jhi // 128)
    return dict(perm=perm, U=U, logitU=logitU, j_rows=j_rows, i_rows=i_rows,
                lo=lo, hi=hi)


def _windows(per_image):
    """Uniform per-slot local-chunk windows (c0[s], W[s]) across cores+images."""
    c0 = np.full(NSLOT, 10 ** 9, np.int64)
    c1 = np.full(NSLOT, -10 ** 9, np.int64)
    for d in per_image:
        for g in range(SHARDS):
            # local chunk = global chunk - 18*(g-1)
            c0 = np.minimum(c0, d["lo"][g] - 18 * (g - 1))
            c1 = np.maximum(c1, d["hi"][g] - 18 * (g - 1))
    c0 = np.maximum(c0, 0)
    c1 = np.minimum(c1, LCHUNKS)
    w = c1 - c0
    assert (w > 0).all() and (c0 >= 0).all() and (c1 <= LCHUNKS).all()
    return tuple(int(v) for v in c0), tuple(int(v) for v in w)


def _core_inputs(data, g):
    """Per-core input tensors. Local j-space = global [2304(g-1), 2304(g+2))."""
    jf = np.zeros((KD, LCHUNKS * 128), np.float32)
    jf[12, :] = -60000.0  # dummy w1: exp -> 0 outside the global range
    glo = max(0, (g - 1) * ROWS)
    ghi = min(N, (g + 2) * ROWS)
    llo = glo - (g - 1) * ROWS
    jf[:, llo:llo + (ghi - glo)] = data["j_rows"][:, glo:ghi]

    iff = data["i_rows"][:, g * ROWS:(g + 1) * ROWS]

    vec = np.zeros((128, 2 * 768), np.float32)
    for s in range(NSLOT):
        sl = slice(g * ROWS + OFF[s], g * ROWS + OFF[s] + TS[s])
        vec[PPART[s], PCOL[s]:PCOL[s] + TS[s]] = data["logitU"][sl]
        vec[PPART[s], 768 + PCOL[s]:768 + PCOL[s] + TS[s]] = data["U"][sl]

    # rows of the padded gather buffer wgp[6]: row c+1 = core c; rows 0/5 = 0
    return {
        "jf": np.ascontiguousarray(jf.astype(BF)),
        "iff": np.ascontiguousarray(iff.astype(BF)),
        "vec": np.ascontiguousarray(vec),
        "idx": np.array([[g], [g + 2]], np.int32),
    }


def build_program(c0, wch):
    import concourse.bacc as bacc
    import concourse.mybir as mybir
    from concourse.tile import TileContext
    from concourse.bass import IndirectOffsetOnAxis

    f32 = mybir.dt.float32
    bf16 = mybir.dt.bfloat16
    fp16 = mybir.dt.float16
    i32 = mybir.dt.int32
    AF = mybir.ActivationFunctionType

    nc = bacc.Bacc(num_devices=N_CORES)

    jf_in = nc.dram_tensor("jf", [KD, LCHUNKS * 128], bf16, kind="ExternalInput")
    if_in = nc.dram_tensor("iff", [KD, ROWS], bf16, kind="ExternalInput")
    vec_in = nc.dram_tensor("vec", [128, 2 * 768], f32, kind="ExternalInput")
    idx_in = nc.dram_tensor("idx", [2, 1], i32, kind="ExternalInput")
    q_out_d = nc.dram_tensor("q_out", [1, ROWS], f32, kind="ExternalOutput")

    SCALE = float(np.float32(1.0) / np.float32(25.0))

    with TileContext(nc) as tc:
        with (
            tc.tile_pool(name="const", bufs=1) as cp,
            tc.tile_pool(name="vp", bufs=2) as vp,
            tc.tile_pool(name="psb", bufs=2, space="PSUM") as pb,
            tc.tile_pool(name="psm", bufs=1, space="PSUM") as pm,
            tc.tile_pool(name="dram", bufs=1, space="DRAM") as dp,
        ):
            # ---------------- persistent SBUF ----------------
            jf_sb = cp.tile([KD, LCHUNKS * 128], bf16, tag="jf")
            if_sb = cp.tile([KD, ROWS], bf16, tag="iff")
            vec_sb = cp.tile([128, 2 * 768], f32, tag="vec")
            logitU = vec_sb[:, 0:768]
            U_sb = vec_sb[:, 768:1536]
            idx_sb = cp.tile([2, 1], i32, tag="idx")
            E_sb = [cp.tile([128, wch[s] * TS[s]], fp16, tag=f"E{s}",
                            name=f"E{s}")
                    for s in range(NSLOT)]
            rsqd = cp.tile([128, 768], f32, tag="rsqd")
            Acoef = cp.tile([128, 768], f32, tag="Acoef")
            Ccoef = cp.tile([128, 768], f32, tag="Ccoef")
            # weight segments: [left, mid, right] x [hi, lo], fp16 [128, 18]
            pseg = [[cp.tile([128, 18], fp16, tag=f"p{side}{hl}",
                             name=f"p{side}{hl}")
                     for hl in range(2)] for side in range(3)]
            # indirect gather staging: row 0 = left neighbor, row 1 = right
            stage = cp.tile([2, 2 * ROWS], fp16, tag="stage")

            nc.sync.dma_start(out=jf_sb[:, :], in_=jf_in[:, :])
            nc.sync.dma_start(out=if_sb[:, :], in_=if_in[:, :])
            nc.sync.dma_start(out=vec_sb[:, :], in_=vec_in[:, :])
            nc.sync.dma_start(out=idx_sb[:, :], in_=idx_in[:, :])

            send = dp.tile([2, ROWS], fp16, tag="send")
            wgp = dp.tile([6, 2, ROWS], fp16, tag="wgp")
            nbr = dp.tile([2, 2 * ROWS], fp16, tag="nbr")

            # zero the padded gather rows (0 and 5) once
            zed = vp.tile([128, 36], fp16, tag="zed")
            nc.vector.memset(zed[:, :], 0.0)
            for row in (0, 5):
                nc.sync.dma_start(out=wgp[row, :, :], in_=zed[:, :])

            # ---------------- build banded E ----------------
            for s in range(NSLOT):
                per = 1536 // TS[s]          # psum chunks per ACT group
                ngrp = -(-wch[s] // per)
                for grp in range(ngrp):
                    ccs = range(grp * per, min(grp * per + per, wch[s]))
                    nk = len(ccs)
                    ps = pb.tile([128, 1536], f32, tag="bps")
                    for k, cc in enumerate(ccs):
                        lc = c0[s] + cc
                        nc.tensor.matmul(
                            ps[:, k * TS[s]:(k + 1) * TS[s]],
                            jf_sb[:, lc * 128:(lc + 1) * 128],
                            if_sb[:, OFF[s]:OFF[s] + TS[s]],
                            start=True, stop=True,
                        )
                    nc.scalar.activation(
                        E_sb[s][:, ccs.start * TS[s]:(ccs.start + nk) * TS[s]],
                        ps[:, 0:nk * TS[s]],
                        AF.Exp, scale=SCALE,
                    )

            # ---------------- mat-vec machinery ----------------
            ps_mv = pm.tile([128, 1024], f32, tag="mv")
            nc.vector.memset(ps_mv[:, :], 1.0)

            def matvec(split):
                """R = E^T w into ps_mv pieces; w from pseg (hi [+lo]).

                Emission is round-robin across slots so the 4 col-tiled
                strips actually run concurrently; within a slot, own (mid)
                chunks go first so the remote-neighbor fetch overlaps."""
                nhl = 2 if split else 1
                orders = [sorted(range(wch[s]),
                                 key=lambda cc: abs(c0[s] + cc - 26))
                          for s in range(NSLOT)]
                emitted = [0] * NSLOT
                for k in range(max(wch)):
                    for s in range(NSLOT):
                        if k >= wch[s]:
                            continue
                        cc = orders[s][k]
                        lc = c0[s] + cc
                        sg, col = pseg[lc // 18], lc % 18
                        out = ps_mv[PPART[s]:PPART[s] + 1,
                                    PCOL[s]:PCOL[s] + TS[s]]
                        nmm = wch[s] * nhl
                        for hl in range(nhl):
                            nc.tensor.matmul(
                                out,
                                sg[hl][:, col:col + 1],
                                E_sb[s][:, cc * TS[s]:(cc + 1) * TS[s]],
                                start=(emitted[s] == 0),
                                stop=(emitted[s] == nmm - 1),
                                tile_position=(0, PPART[s]),
                            )
                            emitted[s] += 1

            def send_pieces(src_h, src_l):
                """piece tiles -> DRAM send rows (j-linear), 10 small DMAs."""
                for s in range(NSLOT):
                    for row, src in ((0, src_h), (1, src_l)):
                        nc.sync.dma_start(
                            out=send[row, OFF[s]:OFF[s] + TS[s]],
                            in_=src[PPART[s]:PPART[s] + 1,
                                    PCOL[s]:PCOL[s] + TS[s]],
                        )

            def distribute():
                """send -> AllGather -> own/mid direct + neighbors indirect."""
                nc.gpsimd.collective_compute(
                    "AllGather", mybir.AluOpType.bypass, replica_groups=RG,
                    ins=[send[:].opt()], outs=[wgp[1:5, :, :].opt()],
                )
                for hl in range(2):
                    nc.sync.dma_start(
                        out=pseg[1][hl][:, :],
                        in_=send[hl, :].rearrange("(c q) -> q c", q=128),
                    )
                nc.gpsimd.indirect_dma_start(
                    out=stage[:, :],
                    out_offset=None,
                    in_=wgp[:, :, :].rearrange("a h c -> a (h c)"),
                    in_offset=IndirectOffsetOnAxis(ap=idx_sb[:, 0:1], axis=0),
                )
                # SBUF->SBUF cannot repartition; bounce via DRAM
                nc.sync.dma_start(out=nbr[:, :], in_=stage[:, :])
                for side in (0, 2):
                    for hl in range(2):
                        nc.sync.dma_start(
                            out=pseg[side][hl][:, :],
                            in_=nbr[side // 2, hl * ROWS:(hl + 1) * ROWS]
                            .rearrange("(c q) -> q c", q=128),
                        )

            def split_send(pf32):
                """f32 piece tile -> fp16 hi/lo pieces + send + distribute."""
                ph = vp.tile([128, 768], fp16, tag="ph")
                pl = vp.tile([128, 768], fp16, tag="pl")
                nc.vector.tensor_copy(ph[:, :], pf32[:, :])
                nc.vector.tensor_sub(pl[:, :], pf32[:, :], ph[:, :])
                send_pieces(ph, pl)
                distribute()

            # ---------------- deg -> rsqd, A ----------------
            for side in range(3):
                nc.vector.memset(pseg[side][0][:, :], 1.0)
                nc.vector.memset(pseg[side][1][:, :], 0.0)
            matvec(split=False)
            lnd = vp.tile([128, 768], f32, tag="t")
            nc.scalar.activation(lnd[:, :], ps_mv[:, 0:768], AF.Ln)
            nc.scalar.activation(rsqd[:, :], lnd[:, :], AF.Exp, scale=-0.5)
            nc.vector.tensor_scalar_mul(Acoef[:, :], rsqd[:, :], 10.0)
            split_send(rsqd)

            # ---------------- tvec -> C ----------------
            matvec(split=True)
            t1 = vp.tile([128, 768], f32, tag="t")
            nc.vector.tensor_mul(t1[:, :], rsqd[:, :], ps_mv[:, 0:768])
            t2 = vp.tile([128, 768], f32, tag="z")
            nc.vector.tensor_scalar_mul(t2[:, :], t1[:, :], -5.0)
            nc.vector.tensor_add(Ccoef[:, :], logitU[:, :], t2[:, :])

            # ---------------- p0 = U * rsqd ----------------
            p0 = vp.tile([128, 768], f32, tag="p")
            nc.vector.tensor_mul(p0[:, :], U_sb[:, :], rsqd[:, :])
            split_send(p0)

            # ---------------- 10 mean-field iterations ----------------
            q = None
            for it in range(REFINE_ITERS):
                matvec(split=True)
                t = vp.tile([128, 768], f32, tag="t")
                nc.vector.tensor_mul(t[:, :], Acoef[:, :], ps_mv[:, 0:768])
                z = vp.tile([128, 768], f32, tag="z")
                nc.vector.tensor_add(z[:, :], Ccoef[:, :], t[:, :])
                q = vp.tile([128, 768], f32, tag="q")
                nc.scalar.activation(q[:, :], z[:, :], AF.Sigmoid)
                if it < REFINE_ITERS - 1:
                    p = vp.tile([128, 768], f32, tag="p")
                    nc.vector.tensor_mul(p[:, :], q[:, :], rsqd[:, :])
                    split_send(p)

            for s in range(NSLOT):
                nc.sync.dma_start(
                    out=q_out_d[0, OFF[s]:OFF[s] + TS[s]],
                    in_=q[PPART[s]:PPART[s] + 1, PCOL[s]:PCOL[s] + TS[s]],
                )

    nc.compile()
    _fix_act_table_loads(nc, mybir)
    return nc


def _fix_act_table_loads(nc, mybir):
    """Point Exp/Ln loads at one shared set; drop same-set reloads."""
    from concourse.hw_specs import get_activation_tables
    AF = mybir.ActivationFunctionType
    tables = list(get_activation_tables(nc.m.arch).items())
    exp_ln = None
    sig = None
    for idx, (_, funcs) in enumerate(tables):
        if exp_ln is None and {AF.Exp, AF.Ln} <= funcs:
            exp_ln = idx
        if sig is None and AF.Sigmoid in funcs:
            sig = idx
    for blk in nc.m.functions[0].blocks:
        il = blk.instructions
        cur = None
        drop = []
        pending = None
        for ins in il:
            tn = type(ins).__name__
            if tn == "InstLoadActFuncSet":
                sync = getattr(ins, "sync_info", None)
                if sync is not None and (sync.on_wait or sync.on_update):
                    cur = None  # unknown state; keep following loads
                    continue
                pending = ins
            elif tn == "InstActivation" and pending is not None:
                f = pending.act_func_set_id
                if ins.func in (AF.Exp, AF.Ln) and exp_ln is not None:
                    f = exp_ln
                elif ins.func == AF.Sigmoid and sig is not None:
                    f = sig
                if f == cur:
                    drop.append(pending)
                else:
                    pending.act_func_set_id = f
                    cur = f
                pending = None
        for ins in drop:
            il.remove(ins)


_NC_CACHE = {}


def make_in_maps(imgs, masks):
    x, m = _host_prep(imgs, masks)
    per_image = [_image_data(x[b], m[b]) for b in range(B)]
    c0, wch = _windows(per_image)
    in_maps = []
    for k in range(N_CORES):
        b, g = divmod(k, SHARDS)
        in_maps.append(_core_inputs(per_image[b], g))
    return in_maps, per_image, c0, wch


def assemble(results, per_image):
    out = np.empty((B, N), np.float32)
    for k in range(N_CORES):
        b, g = divmod(k, SHARDS)
        q = np.asarray(results[k]["q_out"], np.float32).reshape(ROWS)
        perm = per_image[b]["perm"]
        out[b, perm[g * ROWS:(g + 1) * ROWS]] = q
    return out.reshape(B, H, W)


def kernel(imgs, masks):
    from concourse.bass_utils import run_bass_kernel_spmd

    in_maps, per_image, c0, wch = make_in_maps(imgs, masks)
    key = (c0, wch)
    if key not in _NC_CACHE:
        _NC_CACHE[key] = build_program(c0, wch)
    res = run_bass_kernel_spmd(_NC_CACHE[key], in_maps, list(range(N_CORES)))
    return assemble(res.results, per_image)
